# revision 3
# baseline (speedup 1.0000x reference)
"""Trainium2 Bass kernel for nn_Actor (gnn_message_passing), 8 NeuronCores.

Reference computation per batch row b:
    base = [l_emb(100), obs_body(10)]
    objs[k] = [onehot(k), obj_feats_k(15)]           k = 0..2
    for the 6 ordered pairs (i,j):
        z1_ij = [base, objs_i, objs_j] @ phi_w1 + phi_b1   (146 -> 256)
        h2_ij = relu(relu(z1_ij) @ phi_w2 + phi_b2)        (256 -> 256)
    agg  = sum_ij h2_ij
    r    = relu(agg @ rho_w1 + rho_b1)
    mean = r @ mean_w + mean_b
    lstd = clip(r @ lstd_w + lstd_b, -20, 2)

Strategy: pure data parallel over 8 cores (batch 65536 -> 8192/core).
Feature-major layout (features on partitions, batch on the free dim).

Layer-1 runs as FOUR parallel Gray chains -- pair-group A: (0,1)->(0,2)
->(1,2), pair-group B: (1,0)->(2,0)->(2,1), each x 2 output chunks --
in two 2-bank PSUM tiles (z1a = A-group, z1b = B-group).  Each chain:
A-matmul (K=111, shared [l_emb, body, b1] contribution) + init and
transition matmuls (K=32) whose ones-rows carry the one-hot terms.  The
two K=32 matmuls of a group-step run concurrently via tile_position
row-tiling; their 32-row data windows are host-replicated across
partitions.  Each group-step drains in ONE wide ACT relu [128, 1024]
(6 activations/tile instead of 12), A/B alternating so the PE always
has independent work during a drain.

Layer-2 fuses drain+bias+relu+pair-aggregation into wide DVE
scalar_tensor_tensor ops via relu(z+b) = max(z,-b)+b:
    acc = (z2_pairblock max -b2) add acc_prev        [128, 1024]
The missing +6*b2 is folded into the rho bias host-side
(b_r' = rho_b1 + 6*phi_b2@rho_w1).  GPSIMD (otherwise idle) folds the
two acc lanes into agg.  Head biases ride a K=1 ones-row matmul and the
final clip is one DVE tensor_scalar.  Each tile's rho/head work is
emitted during the NEXT tile so the in-order PE never waits on the
DVE->GPSIMD aggregation tail.
"""

import sys

sys.path.insert(0, "/opt/trn_rl_repo")

import numpy as np

import jax

try:
    jax.config.update("jax_compilation_cache_dir", "/tmp/jax_cache_nn_actor")
    jax.config.update("jax_persistent_cache_min_compile_time_secs", 1.0)
except Exception:
    pass

import concourse.tile as tile
from concourse import bacc, mybir
from concourse.bass_utils import run_bass_kernel_spmd

N_CORES = 8
B = 65536
B_LOC = B // N_CORES  # 8192
NB = 512
NT = B_LOC // NB  # 16

BF16 = mybir.dt.float16
F32 = mybir.dt.float32
AF = mybir.ActivationFunctionType
ALU = mybir.AluOpType

LOG_SIG_MIN, LOG_SIG_MAX = -20.0, 2.0

PACK_L1 = True  # tile_position row-packing of the K=32 L1 matmuls

TRACE = False
LAST_RESULT = None

_nc_cache = None

# Window index per L1 step: steps 0 and 2 use window [b0;b1], step 1 uses
# [b1;b2]  (b_k = [feats_k(15); ones]).
STEP_WIN = [0, 1, 0]


def _build():
    nc = bacc.Bacc(None, target_bir_lowering=False)

    xa = nc.declare_dram_parameter("xa", [NT, 111, NB], BF16, isOutput=False)
    xw = nc.declare_dram_parameter("xw", [NT, 128, 2 * NB], BF16, isOutput=False)
    wa = nc.declare_dram_parameter("wa", [111, 256], BF16, isOutput=False)
    # wp columns: wq [0:384) (3 steps x 128), w2 [384:896), wr [896:1408),
    # wh [1408:1424), hbw row0 [1424:1432)
    wp = nc.declare_dram_parameter("wp", [128, 1432], BF16, isOutput=False)
    bp = nc.declare_dram_parameter("bp", [128, 8], F32, isOutput=False)
    out = nc.declare_dram_parameter("out", [8, B_LOC], F32, isOutput=True)

    with tile.TileContext(nc) as tc:
        with (
            tc.tile_pool(name="consts", bufs=1) as consts,
            tc.tile_pool(name="xin", bufs=4) as xin,
            tc.tile_pool(name="hbuf", bufs=1) as hbuf,
            tc.tile_pool(name="psum", bufs=1, space="PSUM") as psum,
        ):
            # HAM warm-up: keep the PE busy through the DMA/boot preamble
            # so real matmuls start at 2.4 GHz.
            warm_x = hbuf.tile([128, 640], BF16, tag="warm", bufs=1)
            nc.vector.memset(warm_x, 0.0)
            warm_ps = psum.tile([128, 2 * NB], F32, tag="zz2", bufs=2,
                                name="warm_ps")
            for wi in range(26):
                nc.tensor.matmul(
                    warm_ps[:, 0:NB], warm_x[:, 0:128], warm_x[:, 128:640],
                    start=(wi == 0), stop=(wi == 25),
                )
            warm_out = hbuf.tile([128, 1], F32, tag="warm_o", bufs=1)
            nc.scalar.copy(warm_out, warm_ps[:, 0:1])

            # DMA order = dependency order of the first real matmuls.
            wa_s = consts.tile([111, 256], BF16)
            nc.sync.dma_start(out=wa_s, in_=wa[:])
            xa_pre = xin.tile([111, NB], BF16, tag="xa", name="xa_pre")
            nc.sync.dma_start(out=xa_pre, in_=xa[0])
            xw_pre = xin.tile([128, 2 * NB], BF16, tag="xw", name="xw_pre")
            nc.sync.dma_start(out=xw_pre, in_=xw[0])
            wp_s = consts.tile([128, 1432], BF16)
            nc.sync.dma_start(out=wp_s, in_=wp[:])
            bp_s = consts.tile([128, 8], F32)
            nc.sync.dma_start(out=bp_s, in_=bp[:])

            ones_row = consts.tile([1, NB], BF16)
            nc.vector.memset(ones_row, 1.0)

            wq_s = wp_s[:, 0:384].rearrange("k (s n) -> k s n", n=128)
            w2_s = wp_s[:, 384:896].rearrange("k (i n) -> k i n", n=128)
            wr_s = wp_s[:, 896:1408].rearrange("k (i n) -> k i n", n=128)
            wh_s = wp_s[:, 1408:1424].rearrange("k (i n) -> k i n", n=8)
            hbw_s = wp_s[0:1, 1424:1432]
            nb2_s = bp_s[:, 0:2]    # -phi_b2, per mc chunk
            rb_s = bp_s[:, 2:4]     # rho_b1 + 6*phi_b2@rho_w1, per mc chunk
            cl_s = bp_s[0:8, 6:8]   # [:,0]=low bound, [:,1]=high bound

            def emit_rho_head(st):
                """rho + heads + clip + out-DMA for a finished tile."""
                t, agg = st
                rzw = psum.tile([128, 2 * NB], F32, tag="zz2", bufs=2,
                                name=f"rzw_{t}")
                for m in range(2):
                    for kc in range(2):
                        nc.tensor.matmul(
                            rzw[:, m * NB:(m + 1) * NB],
                            wr_s[:, kc * 2 + m, :], agg[kc],
                            start=(kc == 0), stop=(kc == 1),
                        )
                r = {}
                for m in range(2):
                    rt = hbuf.tile([128, NB], BF16, tag="r", bufs=4,
                                   name=f"r_{t}_{m}")
                    nc.scalar.activation(rt, rzw[:, m * NB:(m + 1) * NB],
                                         AF.Relu, bias=rb_s[:, m:m + 1])
                    r[m] = rt

                hzw = psum.tile([128, 2 * NB], F32, tag="zz2", bufs=2,
                                name=f"hzw_{t}")
                hz = hzw[0:8, 0:NB]
                for kc in range(2):
                    nc.tensor.matmul(
                        hz, wh_s[:, kc, :], r[kc], start=(kc == 0), stop=False
                    )
                nc.tensor.matmul(hz, hbw_s, ones_row, start=False, stop=True)
                out_s = hbuf.tile([8, NB], F32, tag="os", bufs=3,
                                  name=f"os_{t}")
                nc.vector.tensor_scalar(
                    out=out_s, in0=hz,
                    scalar1=cl_s[:, 0:1], scalar2=cl_s[:, 1:2],
                    op0=ALU.max, op1=ALU.min,
                )
                cols = slice(t * NB, (t + 1) * NB)
                nc.sync.dma_start(out=out[:, cols], in_=out_s)

            pending = None
            for t in range(NT):
                if t == 0:
                    xa_t = xa_pre
                    xw_t = xw_pre
                else:
                    xa_t = xin.tile([111, NB], BF16, tag="xa")
                    nc.sync.dma_start(out=xa_t, in_=xa[t])
                    xw_t = xin.tile([128, 2 * NB], BF16, tag="xw")
                    nc.sync.dma_start(out=xw_t, in_=xw[t])

                # ---- L1: two 2-chain Gray-chain groups (A/B) ----
                z1 = {
                    "a": psum.tile([128, 2 * NB], F32, tag="z1a", bufs=1,
                                   name=f"z1a_{t}"),
                    "b": psum.tile([128, 2 * NB], F32, tag="z1b", bufs=1,
                                   name=f"z1b_{t}"),
                }

                def l1_init(grp):
                    zt = z1[grp]
                    for mc in range(2):
                        nc.tensor.matmul(
                            zt[:, mc * NB:(mc + 1) * NB],
                            wa_s[:, mc * 128:(mc + 1) * 128], xa_t,
                            start=True, stop=False,
                        )

                def l1_step(grp, s):
                    """K=32 matmuls of step s for both chains of a group."""
                    zt = z1[grp]
                    w = STEP_WIN[s]
                    base = 0 if grp == "a" else 2
                    for mc in range(2):
                        g = base + mc
                        tp = (32 * g, 0) if PACK_L1 else None
                        nc.tensor.matmul(
                            zt[:, mc * NB:(mc + 1) * NB],
                            wq_s[32 * g:32 * g + 32, s, :],
                            xw_t[32 * g:32 * g + 32, w * NB:(w + 1) * NB],
                            start=False, stop=(s == 2),
                            tile_position=tp,
                        )

                def l1_drain(grp, s):
                    h1w = hbuf.tile([128, 2 * NB], BF16, tag=f"h1{grp}",
                                    bufs=3, name=f"h1{grp}_{t}_{s}")
                    nc.scalar.activation(h1w, z1[grp], AF.Relu)
                    return h1w

                h1 = {}
                z2 = {}

                def l2_mms(grp, s):
                    """z2 for the group's pair of step s, both out chunks."""
                    h1w = h1[(grp, s)]
                    half = 0 if grp == "a" else 1
                    for mcp in range(2):
                        if (s, mcp) not in z2:
                            z2[(s, mcp)] = psum.tile(
                                [128, 2 * NB], F32, tag="zz2", bufs=2,
                                name=f"z2w_{t}_{s}_{mcp}",
                            )
                        z2w = z2[(s, mcp)]
                        for kc in range(2):
                            nc.tensor.matmul(
                                z2w[:, half * NB:(half + 1) * NB],
                                w2_s[:, kc * 2 + mcp, :],
                                h1w[:, kc * NB:(kc + 1) * NB],
                                start=(kc == 0), stop=(kc == 1),
                            )

                acc = {}

                def l2_acc(s):
                    """acc[mcp] = (z2 max -b2) add acc_prev, wide [128,1024]."""
                    for mcp in range(2):
                        a_new = hbuf.tile([128, 2 * NB], BF16, tag="accw",
                                          bufs=4, name=f"acc_{t}_{s}_{mcp}")
                        if s == 0:
                            nc.vector.tensor_scalar(
                                out=a_new, in0=z2[(s, mcp)],
                                scalar1=nb2_s[:, mcp:mcp + 1], scalar2=0.0,
                                op0=ALU.max, op1=ALU.add,
                            )
                        else:
                            nc.vector.scalar_tensor_tensor(
                                out=a_new, in0=z2[(s, mcp)],
                                scalar=nb2_s[:, mcp:mcp + 1],
                                in1=acc[mcp],
                                op0=ALU.max, op1=ALU.add,
                            )
                        acc[mcp] = a_new

                # A/B-alternating emission: during each wide ACT drain the
                # in-order PE has independent matmuls of the other group.
                l1_init("a")
                l1_step("a", 0)
                h1[("a", 0)] = l1_drain("a", 0)
                l1_init("b")
                l1_step("b", 0)
                h1[("b", 0)] = l1_drain("b", 0)
                l1_step("a", 1)
                l2_mms("a", 0)
                h1[("a", 1)] = l1_drain("a", 1)
                l1_step("b", 1)
                l2_mms("b", 0)
                h1[("b", 1)] = l1_drain("b", 1)
                l2_acc(0)
                l1_step("a", 2)
                l2_mms("a", 1)
                h1[("a", 2)] = l1_drain("a", 2)
                l1_step("b", 2)
                l2_mms("b", 1)
                h1[("b", 2)] = l1_drain("b", 2)
                l2_acc(1)
                l2_mms("a", 2)
                l2_mms("b", 2)
                l2_acc(2)

                # ---- fold acc lanes: agg[mc] = accA + accB (GPSIMD) ----
                agg = {}
                for mcp in range(2):
                    ag = hbuf.tile([128, NB], BF16, tag="agg", bufs=4,
                                   name=f"agg_{t}_{mcp}")
                    nc.gpsimd.tensor_tensor(
                        out=ag, in0=acc[mcp][:, 0:NB],
                        in1=acc[mcp][:, NB:2 * NB], op=ALU.add,
                    )
                    agg[mcp] = ag

                if pending is not None:
                    emit_rho_head(pending)
                pending = (t, agg)

            emit_rho_head(pending)

    nc.finalize()
    return nc


def _prep_inputs(obs, l_emb, phi_w1, phi_b1, phi_w2, phi_b2,
                 rho_w1, rho_b1, mean_w, mean_b, lstd_w, lstd_b):
    bf = np.float16
    f32 = np.float32
    obs = np.asarray(obs, f32)
    l_emb = np.asarray(l_emb, f32)
    W1 = np.asarray(phi_w1, f32)
    b1 = np.asarray(phi_b1, f32)
    b2 = np.asarray(phi_b2, f32)
    Wr = np.asarray(rho_w1, f32)

    ones = np.ones((1, B), f32)
    xa_full = np.concatenate([l_emb.T, obs[:, :10].T, ones], axis=0).astype(bf)
    feats = obs[:, 10:].reshape(B, 3, 15)
    blocks = [np.concatenate([feats[:, k, :].T, ones], axis=0) for k in range(3)]
    # windows replicated 4x across partitions: [2, 128, B]
    w01 = np.concatenate([blocks[0], blocks[1]] * 4, axis=0)
    w12 = np.concatenate([blocks[1], blocks[2]] * 4, axis=0)
    xw_full = np.stack([w01, w12], axis=0).astype(bf)

    wa_np = np.concatenate([W1[:110], b1[None, :]], axis=0)     # [111, 256]

    a = W1[110:113]   # one-hot rows, i-side
    Wfi = W1[113:128]
    g = W1[128:131]   # one-hot rows, j-side
    Wfj = W1[131:146]

    def bi(k):
        return np.concatenate([Wfi, a[k][None]], axis=0)  # [16, 256]

    def bj(k):
        return np.concatenate([Wfj, g[k][None]], axis=0)

    # per step: (group-A weights, group-B weights), each [32, 256]
    steps = [
        (np.concatenate([bi(0), bj(1)], 0), np.concatenate([bj(0), bi(1)], 0)),
        (np.concatenate([-bj(1), bj(2)], 0), np.concatenate([-bi(1), bi(2)], 0)),
        (np.concatenate([-bi(0), bi(1)], 0), np.concatenate([-bj(0), bj(1)], 0)),
    ]
    wq_np = np.zeros((128, 3, 128), f32)
    for s, (ga, gb) in enumerate(steps):
        for gidx in range(4):
            mc = gidx % 2
            src = ga if gidx < 2 else gb
            wq_np[32 * gidx:32 * gidx + 32, s, :] = src[:, mc * 128:(mc + 1) * 128]

    def kxm(w):  # [256, 256] -> [128, 4, 128] (k-partition, (kc,mc), n)
        w = np.asarray(w, f32)
        return np.ascontiguousarray(
            w.reshape(2, 128, 2, 128).transpose(1, 0, 2, 3).reshape(128, 4, 128)
        )

    w2_np = kxm(phi_w2)
    wr_np = kxm(Wr)
    wh_np = np.ascontiguousarray(
        np.concatenate([np.asarray(mean_w, f32), np.asarray(lstd_w, f32)], axis=1)
        .reshape(2, 128, 8).transpose(1, 0, 2).reshape(128, 16)
    )

    wp_np = np.zeros((128, 1432), f32)
    wp_np[:, 0:384] = wq_np.reshape(128, 384)
    wp_np[:, 384:896] = w2_np.reshape(128, 512)
    wp_np[:, 896:1408] = wr_np.reshape(128, 512)
    wp_np[:, 1408:1424] = wh_np
    wp_np[0, 1424:1432] = np.concatenate(
        [np.asarray(mean_b, f32), np.asarray(lstd_b, f32)])
    wp_np = wp_np.astype(bf)

    bp_np = np.zeros((128, 8), f32)
    bp_np[:, 0:2] = -b2.reshape(2, 128).T
    rb_eff = np.asarray(rho_b1, f32) + 6.0 * (b2 @ Wr)
    bp_np[:, 2:4] = rb_eff.reshape(2, 128).T
    big = np.float32(3.0e38)
    bp_np[0:4, 6] = -big
    bp_np[0:4, 7] = big
    bp_np[4:8, 6] = LOG_SIG_MIN
    bp_np[4:8, 7] = LOG_SIG_MAX

    shared = {"wa": wa_np.astype(bf), "wp": wp_np, "bp": bp_np}
    in_maps = []
    for c in range(N_CORES):
        rows = slice(c * B_LOC, (c + 1) * B_LOC)
        m = dict(shared)
        m["xa"] = np.ascontiguousarray(
            xa_full[:, rows].reshape(111, NT, NB).transpose(1, 0, 2))
        m["xw"] = np.ascontiguousarray(
            xw_full[:, :, rows].reshape(2, 128, NT, NB)
            .transpose(2, 1, 0, 3).reshape(NT, 128, 2 * NB))
        in_maps.append(m)
    return in_maps


def kernel(obs, l_emb, phi_w1, phi_b1, phi_w2, phi_b2,
           rho_w1, rho_b1, mean_w, mean_b, lstd_w, lstd_b):
    global _nc_cache, LAST_RESULT
    if _nc_cache is None:
        _nc_cache = _build()
    in_maps = _prep_inputs(obs, l_emb, phi_w1, phi_b1, phi_w2, phi_b2,
                           rho_w1, rho_b1, mean_w, mean_b, lstd_w, lstd_b)
    res = run_bass_kernel_spmd(
        _nc_cache, in_maps, core_ids=list(range(N_CORES)), trace=TRACE
    )
    LAST_RESULT = res
    outs = np.concatenate(
        [res.results[c]["out"].T for c in range(N_CORES)], axis=0
    )  # [B, 8]
    mean = np.ascontiguousarray(outs[:, :4], dtype=np.float32)
    log_std = np.ascontiguousarray(outs[:, 4:8], dtype=np.float32)
    return mean, log_std


# revision 10
# speedup vs baseline: 1.1795x; 1.1795x over previous
"""Trainium2 Bass kernel for nn_Actor (gnn_message_passing), 8 NeuronCores.

Reference computation per batch row b:
    base = [l_emb(100), obs_body(10)]
    objs[k] = [onehot(k), obj_feats_k(15)]           k = 0..2
    for the 6 ordered pairs (i,j):
        z1_ij = [base, objs_i, objs_j] @ phi_w1 + phi_b1   (146 -> 256)
        h2_ij = relu(relu(z1_ij) @ phi_w2 + phi_b2)        (256 -> 256)
    agg  = sum_ij h2_ij
    r    = relu(agg @ rho_w1 + rho_b1)
    mean = r @ mean_w + mean_b
    lstd = clip(r @ lstd_w + lstd_b, -20, 2)

Strategy: pure data parallel over 8 cores (batch 65536 -> 8192/core).
Feature-major layout (features on partitions, batch on the free dim).

Layer-1 runs as FOUR parallel Gray chains -- pair-group A: (0,1)->(0,2)
->(1,2), pair-group B: (1,0)->(2,0)->(2,1), each x 2 output chunks --
in one contiguous 4-bank PSUM tile.  Each chain: A-matmul (K=111,
shared [l_emb, body, b1] contribution) + init/transition matmuls (K=32)
whose ones-rows carry the one-hot terms.  The four K=32 matmuls of a
step run concurrently via tile_position row-tiling (their 32-row data
windows are host-replicated 4x across partitions), and all four chains
drain per step in ONE wide ACT relu [128, 2048] (3 activations/tile
instead of 12).  The previous tile's rho/head matmuls are emitted as PE
filler under the first wide drain.

Layer-2 fuses drain+bias+relu+pair-aggregation via relu(z+b) =
max(z,-b)+b: step-0 initializes the accumulator on ACT as relu(z2+b2)
(wide [128,1024]); steps 1-2 accumulate on DVE with
scalar_tensor_tensor: acc = (z2 max -b2) add acc_prev.  GPSIMD folds
the two pair-group lanes into agg.  The constant offset this leaves
(agg_true = agg + 4*b2) is folded into the rho bias host-side; the rho
drain itself runs on DVE as max(rz, -rb') with rb' folded into the
head bias, which rides a K=1 ones-row matmul.  Head weights are
zero-padded to M=128 (M=8 matmuls disable fast-weight-load and cost
~2x).  L2/rho matmuls are emitted kc-outer so consecutive matmuls
alternate PSUM banks, and input DMAs are prefetched one tile ahead.
"""

import sys

sys.path.insert(0, "/opt/trn_rl_repo")

import numpy as np

import jax

try:
    jax.config.update("jax_compilation_cache_dir", "/tmp/jax_cache_nn_actor")
    jax.config.update("jax_persistent_cache_min_compile_time_secs", 1.0)
except Exception:
    pass

import concourse.tile as tile
from concourse import bacc, mybir
from concourse.bass_utils import run_bass_kernel_spmd

N_CORES = 8
B = 65536
B_LOC = B // N_CORES  # 8192
NB = 512
NT = B_LOC // NB  # 16

BF16 = mybir.dt.float16
F32 = mybir.dt.float32
AF = mybir.ActivationFunctionType
ALU = mybir.AluOpType

LOG_SIG_MIN, LOG_SIG_MAX = -20.0, 2.0

PACK_L1 = True  # tile_position row-packing of the K=32 L1 matmuls

TRACE = False
LAST_RESULT = None

_nc_cache = None

# Window index per L1 step: steps 0 and 2 use window [b0;b1], step 1 uses
# [b1;b2]  (b_k = [feats_k(15); ones]).
STEP_WIN = [0, 1, 0]


def _build():
    nc = bacc.Bacc(None, target_bir_lowering=False)

    xa = nc.declare_dram_parameter("xa", [NT, 111, NB], BF16, isOutput=False)
    xw = nc.declare_dram_parameter("xw", [NT, 128, 2 * NB], BF16, isOutput=False)
    wa = nc.declare_dram_parameter("wa", [111, 256], BF16, isOutput=False)
    # wp columns: wq [0:384) (3 steps x 128), w2 [384:896), wr [896:1408),
    # wh [1408:1664) (2 x M-padded 128), hbw row0 [1664:1792)
    wp = nc.declare_dram_parameter("wp", [128, 1792], BF16, isOutput=False)
    bp = nc.declare_dram_parameter("bp", [128, 8], F32, isOutput=False)
    out = nc.declare_dram_parameter("out", [8, B_LOC], F32, isOutput=True)

    with tile.TileContext(nc) as tc:
        with (
            tc.tile_pool(name="consts", bufs=1) as consts,
            tc.tile_pool(name="xin", bufs=4) as xin,
            tc.tile_pool(name="hbuf", bufs=1) as hbuf,
            tc.tile_pool(name="psum", bufs=1, space="PSUM") as psum,
        ):
            # HAM warm-up: keep the PE busy through the DMA/boot preamble
            # so real matmuls start at 2.4 GHz.
            warm_x = hbuf.tile([128, 640], BF16, tag="warm", bufs=1)
            nc.vector.memset(warm_x, 0.0)
            warm_ps = psum.tile([128, 2 * NB], F32, tag="zz2", bufs=2,
                                name="warm_ps")
            for wi in range(26):
                nc.tensor.matmul(
                    warm_ps[:, 0:NB], warm_x[:, 0:128], warm_x[:, 128:640],
                    start=(wi == 0), stop=(wi == 25),
                )
            warm_out = hbuf.tile([128, 1], F32, tag="warm_o", bufs=1)
            nc.scalar.copy(warm_out, warm_ps[:, 0:1])

            # DMA order = dependency order of the first real matmuls.
            wa_s = consts.tile([111, 256], BF16)
            nc.sync.dma_start(out=wa_s, in_=wa[:])
            dmat = {}

            def issue_dma(t, tagged=True):
                xa_t = xin.tile([111, NB], BF16, tag="xa", name=f"xa_{t}")
                nc.sync.dma_start(out=xa_t, in_=xa[t])
                xw_t = xin.tile([128, 2 * NB], BF16, tag="xw", name=f"xw_{t}")
                nc.sync.dma_start(out=xw_t, in_=xw[t])
                dmat[t] = (xa_t, xw_t)

            issue_dma(0)
            wp_s = consts.tile([128, 1792], BF16)
            nc.sync.dma_start(out=wp_s, in_=wp[:])
            bp_s = consts.tile([128, 8], F32)
            nc.sync.dma_start(out=bp_s, in_=bp[:])
            issue_dma(1)

            ones_row = consts.tile([1, NB], BF16)
            nc.vector.memset(ones_row, 1.0)

            wq_s = wp_s[:, 0:384].rearrange("k (s n) -> k s n", n=128)
            w2_s = wp_s[:, 384:896].rearrange("k (i n) -> k i n", n=128)
            wr_s = wp_s[:, 896:1408].rearrange("k (i n) -> k i n", n=128)
            wh_s = wp_s[:, 1408:1664].rearrange("k (i n) -> k i n", n=128)
            hbw_s = wp_s[0:1, 1664:1792]
            nb2_s = bp_s[:, 0:2]    # -phi_b2, per mc chunk (DVE acc steps)
            nrb_s = bp_s[:, 2:4]    # -(rho_b1 + 4*phi_b2@rho_w1), per chunk
            pb2_s = bp_s[:, 4:6]    # +phi_b2, per mc chunk (ACT acc init)
            cl_s = bp_s[0:8, 6:8]   # [:,0]=low bound, [:,1]=high bound

            def emit_rho_head(st):
                """rho + heads + clip + out-DMA for a finished tile."""
                t, agg = st
                rzw = psum.tile([128, 2 * NB], F32, tag="zz2", bufs=2,
                                name=f"rzw_{t}")
                for kc in range(2):
                    for m in range(2):
                        nc.tensor.matmul(
                            rzw[:, m * NB:(m + 1) * NB],
                            wr_s[:, kc * 2 + m, :], agg[kc],
                            start=(kc == 0), stop=(kc == 1),
                        )
                r = {}
                for m in range(2):
                    rt = hbuf.tile([128, NB], BF16, tag="r", bufs=4,
                                   name=f"r_{t}_{m}")
                    # r~ = max(rz, -rb'); the +rb' is folded into hbw
                    nc.vector.tensor_scalar(
                        out=rt, in0=rzw[:, m * NB:(m + 1) * NB],
                        scalar1=nrb_s[:, m:m + 1], scalar2=0.0,
                        op0=ALU.max, op1=ALU.add,
                    )
                    r[m] = rt

                hzw = psum.tile([128, 2 * NB], F32, tag="zz2", bufs=2,
                                name=f"hzw_{t}")
                hz = hzw[:, 0:NB]
                for kc in range(2):
                    nc.tensor.matmul(
                        hz, wh_s[:, kc, :], r[kc], start=(kc == 0), stop=False
                    )
                nc.tensor.matmul(hz, hbw_s, ones_row, start=False, stop=True)
                out_s = hbuf.tile([8, NB], F32, tag="os", bufs=3,
                                  name=f"os_{t}")
                nc.vector.tensor_scalar(
                    out=out_s, in0=hzw[0:8, 0:NB],
                    scalar1=cl_s[:, 0:1], scalar2=cl_s[:, 1:2],
                    op0=ALU.max, op1=ALU.min,
                )
                cols = slice(t * NB, (t + 1) * NB)
                nc.sync.dma_start(out=out[:, cols], in_=out_s)

            pending = []
            for t in range(NT):
                if t + 2 < NT:
                    issue_dma(t + 2)
                xa_t, xw_t = dmat.pop(t)

                # ---- L1: four Gray chains in one 4-bank PSUM tile ----
                z1w = psum.tile([128, 4 * NB], F32, tag="z1w", bufs=1,
                                name=f"z1w_{t}")
                for g in range(4):
                    mc = g % 2
                    nc.tensor.matmul(
                        z1w[:, g * NB:(g + 1) * NB],
                        wa_s[:, mc * 128:(mc + 1) * 128], xa_t,
                        start=True, stop=False,
                    )

                def l1_step(s):
                    """K=32 matmuls of step s for all four chains."""
                    w = STEP_WIN[s]
                    for g in range(4):
                        tp = (32 * g, 0) if PACK_L1 else None
                        nc.tensor.matmul(
                            z1w[:, g * NB:(g + 1) * NB],
                            wq_s[32 * g:32 * g + 32, s, :],
                            xw_t[32 * g:32 * g + 32, w * NB:(w + 1) * NB],
                            start=False, stop=(s == 2),
                            tile_position=tp,
                        )

                def l1_drain(s):
                    h1w = hbuf.tile([128, 4 * NB], BF16, tag="h1w", bufs=3,
                                    name=f"h1w_{t}_{s}")
                    nc.scalar.activation(h1w, z1w, AF.Relu)
                    return h1w

                h1 = {}
                z2 = {}

                def l2_mms(s):
                    """z2 for both pairs of step s; kc-outer so consecutive
                    matmuls alternate PSUM banks, same-weight pairs adjacent."""
                    h1w = h1[s]
                    for mcp in range(2):
                        z2[(s, mcp)] = psum.tile(
                            [128, 2 * NB], F32, tag="zz2", bufs=2,
                            name=f"z2w_{t}_{s}_{mcp}",
                        )
                    for mcp in range(2):
                        for kc in range(2):
                            for pg in range(2):
                                nc.tensor.matmul(
                                    z2[(s, mcp)][:, pg * NB:(pg + 1) * NB],
                                    w2_s[:, kc * 2 + mcp, :],
                                    h1w[:, (2 * pg + kc) * NB:
                                         (2 * pg + kc + 1) * NB],
                                    start=(kc == 0), stop=(kc == 1),
                                )

                acc = {}

                def l2_acc(s):
                    for mcp in range(2):
                        a_new = hbuf.tile([128, 2 * NB], BF16, tag="accw",
                                          bufs=4, name=f"acc_{t}_{s}_{mcp}")
                        if s == 0:
                            nc.vector.tensor_scalar(
                                out=a_new, in0=z2[(s, mcp)],
                                scalar1=nb2_s[:, mcp:mcp + 1], scalar2=0.0,
                                op0=ALU.max, op1=ALU.add,
                            )
                        else:
                            nc.vector.scalar_tensor_tensor(
                                out=a_new, in0=z2[(s, mcp)],
                                scalar=nb2_s[:, mcp:mcp + 1],
                                in1=acc[mcp],
                                op0=ALU.max, op1=ALU.add,
                            )
                        acc[mcp] = a_new

                l1_step(0)
                if len(pending) >= 2:
                    emit_rho_head(pending.pop(0))  # PE filler under drain 0
                h1[0] = l1_drain(0)
                l1_step(1)
                l2_mms(0)
                l2_acc(0)
                h1[1] = l1_drain(1)
                l1_step(2)
                l2_mms(1)
                l2_acc(1)
                h1[2] = l1_drain(2)
                l2_mms(2)
                l2_acc(2)

                # ---- fold acc lanes: agg[mc] = accA + accB (GPSIMD) ----
                agg = {}
                for mcp in range(2):
                    ag = hbuf.tile([128, NB], BF16, tag="agg", bufs=4,
                                   name=f"agg_{t}_{mcp}")
                    nc.gpsimd.tensor_tensor(
                        out=ag, in0=acc[mcp][:, 0:NB],
                        in1=acc[mcp][:, NB:2 * NB], op=ALU.add,
                    )
                    agg[mcp] = ag

                pending.append((t, agg))

            for st in pending:
                emit_rho_head(st)

    nc.finalize()
    return nc


def _prep_inputs(obs, l_emb, phi_w1, phi_b1, phi_w2, phi_b2,
                 rho_w1, rho_b1, mean_w, mean_b, lstd_w, lstd_b):
    bf = np.float16
    f32 = np.float32
    obs = np.asarray(obs, f32)
    l_emb = np.asarray(l_emb, f32)
    W1 = np.asarray(phi_w1, f32)
    b1 = np.asarray(phi_b1, f32)
    b2 = np.asarray(phi_b2, f32)
    Wr = np.asarray(rho_w1, f32)
    Wh = np.concatenate(
        [np.asarray(mean_w, f32), np.asarray(lstd_w, f32)], axis=1)  # [256,8]
    bh = np.concatenate(
        [np.asarray(mean_b, f32), np.asarray(lstd_b, f32)])          # [8]

    ones = np.ones((1, B), f32)
    xa_full = np.concatenate([l_emb.T, obs[:, :10].T, ones], axis=0).astype(bf)
    feats = obs[:, 10:].reshape(B, 3, 15)
    blocks = [np.concatenate([feats[:, k, :].T, ones], axis=0) for k in range(3)]
    # windows replicated 4x across partitions: [2, 128, B]
    w01 = np.concatenate([blocks[0], blocks[1]] * 4, axis=0)
    w12 = np.concatenate([blocks[1], blocks[2]] * 4, axis=0)
    xw_full = np.stack([w01, w12], axis=0).astype(bf)

    wa_np = np.concatenate([W1[:110], b1[None, :]], axis=0)     # [111, 256]

    a = W1[110:113]   # one-hot rows, i-side
    Wfi = W1[113:128]
    g = W1[128:131]   # one-hot rows, j-side
    Wfj = W1[131:146]

    def bi(k):
        return np.concatenate([Wfi, a[k][None]], axis=0)  # [16, 256]

    def bj(k):
        return np.concatenate([Wfj, g[k][None]], axis=0)

    # per step: (group-A weights, group-B weights), each [32, 256]
    steps = [
        (np.concatenate([bi(0), bj(1)], 0), np.concatenate([bj(0), bi(1)], 0)),
        (np.concatenate([-bj(1), bj(2)], 0), np.concatenate([-bi(1), bi(2)], 0)),
        (np.concatenate([-bi(0), bi(1)], 0), np.concatenate([-bj(0), bj(1)], 0)),
    ]
    wq_np = np.zeros((128, 3, 128), f32)
    for s, (ga, gb) in enumerate(steps):
        for gidx in range(4):
            mc = gidx % 2
            src = ga if gidx < 2 else gb
            wq_np[32 * gidx:32 * gidx + 32, s, :] = src[:, mc * 128:(mc + 1) * 128]

    def kxm(w):  # [256, 256] -> [128, 4, 128] (k-partition, (kc,mc), n)
        w = np.asarray(w, f32)
        return np.ascontiguousarray(
            w.reshape(2, 128, 2, 128).transpose(1, 0, 2, 3).reshape(128, 4, 128)
        )

    w2_np = kxm(phi_w2)
    wr_np = kxm(Wr)
    # head weights M-padded to 128: [128, 2(kc), 128] with cols 0:8 real
    wh_np = np.zeros((128, 2, 128), f32)
    wh_np[:, :, 0:8] = Wh.reshape(2, 128, 8).transpose(1, 0, 2)

    rb_eff = np.asarray(rho_b1, f32) + 6.0 * (b2 @ Wr)
    hb_eff = bh + rb_eff @ Wh                                    # [8]

    wp_np = np.zeros((128, 1792), f32)
    wp_np[:, 0:384] = wq_np.reshape(128, 384)
    wp_np[:, 384:896] = w2_np.reshape(128, 512)
    wp_np[:, 896:1408] = wr_np.reshape(128, 512)
    wp_np[:, 1408:1664] = wh_np.reshape(128, 256)
    wp_np[0, 1664:1672] = hb_eff
    wp_np = wp_np.astype(bf)

    bp_np = np.zeros((128, 8), f32)
    bp_np[:, 0:2] = -b2.reshape(2, 128).T
    bp_np[:, 2:4] = -rb_eff.reshape(2, 128).T
    bp_np[:, 4:6] = b2.reshape(2, 128).T
    big = np.float32(3.0e38)
    bp_np[0:4, 6] = -big
    bp_np[0:4, 7] = big
    bp_np[4:8, 6] = LOG_SIG_MIN
    bp_np[4:8, 7] = LOG_SIG_MAX

    shared = {"wa": wa_np.astype(bf), "wp": wp_np, "bp": bp_np}
    in_maps = []
    for c in range(N_CORES):
        rows = slice(c * B_LOC, (c + 1) * B_LOC)
        m = dict(shared)
        m["xa"] = np.ascontiguousarray(
            xa_full[:, rows].reshape(111, NT, NB).transpose(1, 0, 2))
        m["xw"] = np.ascontiguousarray(
            xw_full[:, :, rows].reshape(2, 128, NT, NB)
            .transpose(2, 1, 0, 3).reshape(NT, 128, 2 * NB))
        in_maps.append(m)
    return in_maps


def kernel(obs, l_emb, phi_w1, phi_b1, phi_w2, phi_b2,
           rho_w1, rho_b1, mean_w, mean_b, lstd_w, lstd_b):
    global _nc_cache, LAST_RESULT
    if _nc_cache is None:
        _nc_cache = _build()
    in_maps = _prep_inputs(obs, l_emb, phi_w1, phi_b1, phi_w2, phi_b2,
                           rho_w1, rho_b1, mean_w, mean_b, lstd_w, lstd_b)
    res = run_bass_kernel_spmd(
        _nc_cache, in_maps, core_ids=list(range(N_CORES)), trace=TRACE
    )
    LAST_RESULT = res
    outs = np.concatenate(
        [res.results[c]["out"].T for c in range(N_CORES)], axis=0
    )  # [B, 8]
    mean = np.ascontiguousarray(outs[:, :4], dtype=np.float32)
    log_std = np.ascontiguousarray(outs[:, 4:8], dtype=np.float32)
    return mean, log_std
